# revision 1
# baseline (speedup 1.0000x reference)
"""Fused CNN-LSTM cell (locked dropout) Trainium2 kernel.

Math (per row b of a batch of B):
    concat = [x_t, h_prev] * mask[b]          # [B, 128]
    gates  = concat @ [W_i|W_f|W_o|W_g] + b   # [B, 256]
    i,f,o  = sigmoid(gates[:, :192]);  g = tanh(gates[:, 192:])
    c      = f * c_prev + i * g
    h      = o * tanh(c)
    returns (h, c)

Distribution: data-parallel over the batch dim across 8 NeuronCores
(32768 rows/core); gate weights replicated.

Per-core dataflow (macro = 32 chunks of 128 rows, software-pipelined with
stage lags so no in-order engine stream ever waits on a later pipeline
stage):
    - Pool/SWDGE cast-DMA prefetch of x/h/c_prev fp32->bf16, batch-major
      [128 part = row-in-chunk, chunk, feat]; this stream only reads DRAM
      so it never stalls
    - rows are quad-packed (row = k*512 + 4p + q) so every DRAM run covers
      four consecutive rows: 512B bf16 cast-load / 1KB fp32 store
      descriptors at full DMA width
    - DVE tensor_scalar multiplies x and h planes of each chunk by the
      per-row dropout mask (exact fp32 mask, loaded once in the same
      quad-packed layout)
    - ONE xbar DMA-transpose per macro ([128, 4096] bf16 -> [128, 32, 128])
      turns each [row, feat] 128x128 block into [feat, row] so the feature
      dim lands on partitions for the matmuls
    - PE: gates[128,256] = catT.T @ W (bf16, fp32 psum); bias added with a
      K=1 accumulating matmul (ones[1,128].T @ [b|b][1,512] per bank);
      exactly one start=True per 2KB PSUM bank (bank-wide has_written clear)
    - ACT (lag 1): sigmoid over i|f|o columns, tanh over g -> bf16
    - DVE (lag 2): f*c_prev, i*g, add (bf16 2x mode); ACT: tanh(c)
    - DVE (lag 3): o*tanh(c) and fp32 widening; fp32 stores via SP/HWDGE
"""

import numpy as np

from concourse import bacc, mybir, tile
from concourse.bass_utils import run_bass_kernel_spmd

B, D, H = 262144, 64, 64
N_CORES = 8
B_LOC = B // N_CORES  # 32768
CHUNK = 128           # rows per matmul tile (partition dim)
MACRO = 32            # chunks per macro-iteration

F32 = mybir.dt.float32
BF16 = mybir.dt.bfloat16
GATE_ORDER = ("i", "f", "o", "g")


def build_bass(b_loc: int = B_LOC, load_bufs: int = 4, work_bufs: int = 2,
               n_transp: int = 1, gate_group: int = 8, psum_bufs: int = 2,
               prefetch: int = 2, ts_on_act: int = 0):
    assert b_loc % (CHUNK * MACRO) == 0
    n_chunks = b_loc // CHUNK
    assert n_chunks % 16 == 0  # xbar-transpose partition granularity for the mask
    n_macro = b_loc // (CHUNK * MACRO)

    nc = bacc.Bacc("TRN2", target_bir_lowering=False, debug=False)

    x_d = nc.dram_tensor("x_t", [b_loc, D], F32, kind="ExternalInput")
    h_d = nc.dram_tensor("h_prev", [b_loc, H], F32, kind="ExternalInput")
    c_d = nc.dram_tensor("c_prev", [b_loc, H], F32, kind="ExternalInput")
    m_d = nc.dram_tensor("mask", [b_loc, 1], F32, kind="ExternalInput")
    w_d = {g: nc.dram_tensor(f"W_{g}", [D + H, H], F32, kind="ExternalInput")
           for g in GATE_ORDER}
    b_d = {g: nc.dram_tensor(f"b_{g}", [1, H], F32, kind="ExternalInput")
           for g in GATE_ORDER}
    ho_d = nc.dram_tensor("h_out", [b_loc, H], F32, kind="ExternalOutput")
    co_d = nc.dram_tensor("c_out", [b_loc, H], F32, kind="ExternalOutput")

    # Quad-packed batch views: row = k*512 + 4p + q, i.e. partition p of
    # block (k, q) holds row k*512+4p+q. DRAM runs over (q, f) are 1KB fp32
    # / 512B bf16 (four consecutive rows), so both the cast-loads and the
    # fp32 stores get full-width descriptors. Kernel-internal "chunk" index
    # c = 4k + q; the math is row-permutation invariant as long as every
    # view (incl. the mask) uses the same map.
    Q = 4
    xv = x_d[:].rearrange("(k p q) f -> p k q f", p=CHUNK, q=Q)
    hv = h_d[:].rearrange("(k p q) f -> p k q f", p=CHUNK, q=Q)
    cv = c_d[:].rearrange("(k p q) f -> p k q f", p=CHUNK, q=Q)
    hov = ho_d[:].rearrange("(k p q) f -> p k q f", p=CHUNK, q=Q)
    cov = co_d[:].rearrange("(k p q) f -> p k q f", p=CHUNK, q=Q)
    # mask in the same quad-packed layout: [128, n_chunks//Q, Q]
    mv = m_d[:].rearrange("(k p q) one -> p k (q one)", p=CHUNK, q=Q)

    with tile.TileContext(nc) as tc:
        with tc.tile_pool(name="const", bufs=1) as constp, \
             tc.tile_pool(name="loads", bufs=load_bufs) as loadp, \
             tc.tile_pool(name="work", bufs=work_bufs) as workp:

            # ---- one-time constants ----
            w_bf = constp.tile([D + H, 4 * H], BF16)     # [128, 256]
            b2_bf = constp.tile([1, 2 * 4 * H], BF16)    # bias repeated twice: [1, 512]
            ones_bf = constp.tile([1, CHUNK], BF16)
            # quad-packed per-row dropout mask: mask_pm[p, k, q] = mask of
            # row k*512+4p+q; viewed flat as [128, n_chunks] with chunk
            # index c = 4k+q (matches the compute chunk indexing)
            mask_pm = constp.tile([CHUNK, n_chunks // 4, 4], F32)
            mask_cm = mask_pm[:].rearrange("p k q -> p (k q)")

            def load_consts(first_chunks):
                # first-macro mask slice first: it gates TS(0); the bulk of
                # the mask and the weights can land under the data prefetch
                nc.sync.dma_start(mask_pm[:, 0:first_chunks // 4, :],
                                  mv[:, 0:first_chunks // 4, :])
                for gi, g in enumerate(GATE_ORDER):
                    nc.gpsimd.dma_start(w_bf[:, gi * H:(gi + 1) * H], w_d[g][:])
                    for rep in range(2):
                        nc.gpsimd.dma_start(
                            b2_bf[:, rep * 4 * H + gi * H:
                                  rep * 4 * H + (gi + 1) * H],
                            b_d[g][:])
                nc.vector.memset(ones_bf[:], 1.0)
                nc.sync.dma_start(mask_pm[:, first_chunks // 4:, :],
                                  mv[:, first_chunks // 4:, :])

            # ---- main loop, software-pipelined over macros ----
            # Engine-stream discipline (each DMA-issuing sequencer is
            # in-order, so a stream must never mix late-stage waits ahead of
            # early-stage work):
            #   gpsimd (Pool): prefetch cast-loads only (never waits)
            #   sync (SP):     the per-macro transpose + fp32 stores
            #   scalar (ACT):  activations only
            PREFETCH = prefetch
            psump = tc.alloc_tile_pool(name="psum", bufs=2, space="PSUM")
            stash = {}
            loaded = {}

            def issue_loads(m):
                # all prefetch loads on gpsimd/SWDGE with fp32->bf16 cast:
                # the Pool stream reads only DRAM (never waits), so it can
                # run arbitrarily far ahead. x and h land in separate planes
                # of one tile so each is a single full-descriptor-width DMA
                # (the (q f) runs are 512B bf16 / 1KB fp32).
                ksp = slice(m * MACRO // 4, (m + 1) * MACRO // 4)
                xh = loadp.tile([CHUNK, 2, MACRO // 4, 4, D], BF16, tag="catm")
                nc.gpsimd.dma_start(xh[:, 0, :, :, :], xv[:, ksp, :, :])
                nc.gpsimd.dma_start(xh[:, 1, :, :, :], hv[:, ksp, :, :])
                # cpb lives from prefetch until stage_b two macros later
                cpb = loadp.tile([CHUNK, MACRO, H], BF16, tag="cpb", bufs=5)
                nc.gpsimd.dma_start(
                    cpb.rearrange("p (k q) f -> p k q f", q=4), cv[:, ksp, :, :])
                loaded[m] = (xh, cpb)

            def stage_a(m):
                xh, cpb = loaded.pop(m)
                # mask-mul (x and h planes of chunk c in one strided op),
                # then transposes in n_transp batches; matmuls in
                # gate_group-chunk PSUM groups (gate_group/2 banks each)
                catms = workp.tile([CHUNK, MACRO, D + H], BF16, tag="catms")
                catT = workp.tile([D + H, MACRO, CHUNK], BF16, tag="catT")
                gates_groups = []
                tspan = MACRO // n_transp
                for h in range(n_transp):
                    for k in range(h * tspan, (h + 1) * tspan):
                        kk = m * MACRO + k
                        kq, qq = divmod(k, 4)
                        nc.vector.tensor_scalar_mul(
                            catms[:, k, :].rearrange("p (pl f) -> p pl f", pl=2),
                            xh[:, :, kq, qq, :],
                            mask_cm[:, kk:kk + 1])
                    hs = slice(h * tspan, (h + 1) * tspan)
                    nc.sync.dma_start_transpose(catT[:, hs, :], catms[:, hs, :])
                    # one accumulation group per 2KB PSUM bank (2 chunks/bank):
                    # start=True clears has_written bank-wide, so it appears
                    # exactly once per bank, before everything else in it
                    for g in range(tspan // gate_group):
                        gates = psump.tile([CHUNK, gate_group, 4 * H], F32,
                                           tag="gates", bufs=psum_bufs)
                        for kb in range(gate_group // 2):
                            k0 = h * tspan + g * gate_group + 2 * kb
                            nc.tensor.matmul(gates[:, 2 * kb, :], catT[:, k0, :],
                                             w_bf[:], start=True, stop=False)
                            nc.tensor.matmul(gates[:, 2 * kb + 1, :],
                                             catT[:, k0 + 1, :],
                                             w_bf[:], start=False, stop=False)
                            nc.tensor.matmul(
                                gates[:, 2 * kb:2 * kb + 2, :].rearrange(
                                    "p a b -> p (a b)"),
                                ones_bf[:], b2_bf[:],
                                start=False, stop=True, skip_group_check=True)
                        gates_groups.append(gates)

                stash[m] = (gates_groups, cpb)

            def stage_act(m):
                gates_groups, cpb = stash.pop(m)
                ifo = workp.tile([CHUNK, MACRO, 3 * H], BF16, tag="ifo")
                gt = workp.tile([CHUNK, MACRO, H], BF16, tag="gt")
                gg = MACRO // len(gates_groups)
                for q, gates in enumerate(gates_groups):
                    qs = slice(q * gg, (q + 1) * gg)
                    nc.scalar.activation(ifo[:, qs, :], gates[:, :, 0:3 * H],
                                         mybir.ActivationFunctionType.Sigmoid)
                    nc.scalar.activation(gt[:, qs, :], gates[:, :, 3 * H:4 * H],
                                         mybir.ActivationFunctionType.Tanh)
                stash[m] = (ifo, gt, cpb)

            def stage_b1(m):
                # c = f*c_prev + i*g on DVE (fp32 out, ready for storing),
                # tanh(c) on ACT; the c-store goes out right here (lag 2)
                ksp = slice(m * MACRO // 4, (m + 1) * MACRO // 4)
                ifo, gt, cpb = stash.pop(m)
                t1 = workp.tile([CHUNK, MACRO, H], BF16, tag="t1")
                t2 = workp.tile([CHUNK, MACRO, H], BF16, tag="t2")
                cf = workp.tile([CHUNK, MACRO, H], F32, tag="cf", bufs=3)
                th = workp.tile([CHUNK, MACRO, H], BF16, tag="th", bufs=4)
                nc.vector.tensor_mul(t1[:], ifo[:, :, H:2 * H], cpb[:])   # f * c_prev
                nc.vector.tensor_mul(t2[:], ifo[:, :, 0:H], gt[:])        # i * g
                nc.vector.tensor_add(cf[:], t1[:], t2[:])                 # c (fp32)
                nc.scalar.activation(th[:], cf[:],
                                     mybir.ActivationFunctionType.Tanh)
                nc.sync.dma_start(cov[:, ksp, :, :],
                                  cf.rearrange("p (k q) f -> p k q f", q=4))
                stash[("b", m)] = (ifo, th)

            def stage_b2(m):
                # h = o*tanh(c) widened to fp32, stored via HWDGE (SP)
                ksp = slice(m * MACRO // 4, (m + 1) * MACRO // 4)
                ifo, th = stash.pop(("b", m))
                hf = workp.tile([CHUNK, MACRO, H], F32, tag="hf")
                nc.vector.tensor_mul(hf[:], ifo[:, :, 2 * H:3 * H], th[:])
                nc.sync.dma_start(hov[:, ksp, :, :],
                                  hf.rearrange("p (k q) f -> p k q f", q=4))

            for m in range(min(PREFETCH, n_macro)):
                issue_loads(m)
            load_consts(first_chunks=2 * MACRO)
            for m in range(n_macro + 3):
                if m + PREFETCH < n_macro:
                    issue_loads(m + PREFETCH)
                if m < n_macro:
                    stage_a(m)
                if 1 <= m <= n_macro:
                    stage_act(m - 1)
                if 2 <= m <= n_macro + 1:
                    stage_b1(m - 2)
                if m >= 3:
                    stage_b2(m - 3)

            psump.release()

    nc.compile()
    return nc


_CACHED_NC = None


def _get_nc():
    global _CACHED_NC
    if _CACHED_NC is None:
        _CACHED_NC = build_bass(B_LOC)
    return _CACHED_NC


def make_in_maps(inputs: dict, b_loc: int = B_LOC, n_cores: int = N_CORES):
    in_maps = []
    for c in range(n_cores):
        sl = slice(c * b_loc, (c + 1) * b_loc)
        im = {
            "x_t": np.ascontiguousarray(inputs["x_t"][sl], dtype=np.float32),
            "h_prev": np.ascontiguousarray(inputs["h_prev"][sl], dtype=np.float32),
            "c_prev": np.ascontiguousarray(inputs["c_prev"][sl], dtype=np.float32),
            "mask": np.ascontiguousarray(inputs["mask"][sl], dtype=np.float32),
        }
        for g in GATE_ORDER:
            im[f"W_{g}"] = np.ascontiguousarray(inputs[f"W_{g}"], dtype=np.float32)
            im[f"b_{g}"] = np.ascontiguousarray(
                np.asarray(inputs[f"b_{g}"], dtype=np.float32).reshape(1, H))
        in_maps.append(im)
    return in_maps


def kernel(**inputs):
    nc = _get_nc()
    in_maps = make_in_maps(inputs)
    res = run_bass_kernel_spmd(nc, in_maps, core_ids=list(range(N_CORES)))
    h = np.concatenate([res.results[c]["h_out"] for c in range(N_CORES)], axis=0)
    c = np.concatenate([res.results[c]["c_out"] for c in range(N_CORES)], axis=0)
    return (h, c)

